# revision 13
# baseline (speedup 1.0000x reference)
"""Multi-head causal attention (B=4, T=2048, D=1024, 16 heads) on 8 TRN2 cores.

Sharding: core c -> batch b = c//2, head-group g = c%2 (8 of 16 heads).
Each core computes its batch's QKV for its heads, flash-style causal
attention with scores kept transposed (S^T[k, q]; softmax sums come free
via a ones-column appended to V), then a partial output projection
y_part = attn_local @ W_proj[rows]. Host sums the two head-group partials
per batch.

v2 structure: x arrives pre-transposed (host does it for free), all
weights are SBUF-resident, and the per-t-chunk QKV work is emitted
interleaved with the previous q-chunk's attention so the Scalar engine
(exp) and PE overlap across the whole kernel instead of phase-by-phase.
Per head, a wave of score blocks is emitted before the wave's AV
accumulations so the exp latency hides behind the next scores matmul.
Score/exp/mask/AV work is sliced to the live (causal) columns of each
block. Softmax normalization: per-head l row -> reciprocal_approx_fast
-> gpsimd partition_broadcast -> one fused multiply into attnT.
"""

import math
from contextlib import ExitStack

import numpy as np

import concourse.bacc as bacc
import concourse.bass as bass
import concourse.mybir as mybir
import concourse.tile as tile
from concourse.bass_utils import run_bass_kernel_spmd

AF = mybir.ActivationFunctionType
F32 = mybir.dt.float32
F16 = mybir.dt.float16

B_FULL = 4
T_FULL = 2048
D_FULL = 1024
NH_FULL = 16
HD = 64
WAVE = 8  # score blocks per exp/AV wave (bounds live pT tiles)


def build_program(T, D, HL, n_pat, blocks):
    """blocks[qc] = list of (ki, b_live, pat) for active S^T blocks
    [k-tile ki, q-chunk qc]; b_live = first live column within the
    chunk (cols < b_live are fully masked), pat = None | ("tri", base)
    | ("pat", idx)."""
    CL = HL * HD            # local channels (512)
    NDT = D // 128          # contraction tiles for qkv matmuls (8)
    NTT = T // 128          # t-tiles (16)
    QCW = min(512, T)       # q-chunk width
    NQC = T // QCW          # q-chunks (4)
    TPC = QCW // 128        # t-tiles per chunk (4)
    NCT = CL // 128         # c-tiles for q/k/attn storage (4)
    PCH = min(512, D)       # proj output chunk
    NPCH = D // PCH
    scale = 1.0 / math.sqrt(HD)

    nc = bacc.Bacc("TRN2", target_bir_lowering=False, debug=False)
    xT = nc.dram_tensor("xT", [D, T], F16, kind="ExternalInput").ap()
    wq = nc.dram_tensor("wq", [D, CL], F16, kind="ExternalInput").ap()
    wk = nc.dram_tensor("wk", [D, CL], F16, kind="ExternalInput").ap()
    wv = nc.dram_tensor("wv", [D, CL], F16, kind="ExternalInput").ap()
    bq = nc.dram_tensor("bq", [CL], F32, kind="ExternalInput").ap()
    bk = nc.dram_tensor("bk", [CL], F32, kind="ExternalInput").ap()
    bv = nc.dram_tensor("bv", [CL], F32, kind="ExternalInput").ap()
    wp = nc.dram_tensor("wp", [CL, D], F16, kind="ExternalInput").ap()
    bp = nc.dram_tensor("bp", [D], F32, kind="ExternalInput").ap()
    mp = nc.dram_tensor("mp", [max(n_pat, 1), 128, QCW], F16, kind="ExternalInput").ap()
    y = nc.dram_tensor("y", [T, D], F32, kind="ExternalOutput").ap()

    with tile.TileContext(nc) as tc, nc.allow_low_precision(
        reason="fp16 operands carry ~1e-3 relative error; tolerance is 2e-2"
    ), ExitStack() as ctx:
        persist = ctx.enter_context(tc.tile_pool(name="persist", bufs=1))
        xtp = ctx.enter_context(tc.tile_pool(name="xtp", bufs=2))
        pqp = ctx.enter_context(tc.tile_pool(name="pqp", bufs=2, space="PSUM"))
        psp = ctx.enter_context(tc.tile_pool(name="psp", bufs=3, space="PSUM"))
        pvp = ctx.enter_context(tc.tile_pool(name="pvp", bufs=3, space="PSUM"))
        ptl = ctx.enter_context(tc.tile_pool(name="ptl", bufs=WAVE + 2))
        rip = ctx.enter_context(tc.tile_pool(name="rip", bufs=3))
        rbp = ctx.enter_context(tc.tile_pool(name="rbp", bufs=3))
        ysb = ctx.enter_context(tc.tile_pool(name="ysb", bufs=3))

        # ---- persistent state ----
        kT = [persist.tile([128, T], F16, name=f"kT{i}", tag=f"kT{i}") for i in range(NCT)]
        # Q^T stored twice, zero-padded per head parity: K<=64 matmuls
        # stream at half rate, so scores contract over all 128 partitions
        # with zero rows killing the other head's channels. The zeroing
        # memsets are emitted per chunk inside the pipeline.
        qTe = [persist.tile([128, T], F16, name=f"qTe{i}", tag=f"qTe{i}") for i in range(NCT)]
        qTo = [persist.tile([128, T], F16, name=f"qTo{i}", tag=f"qTo{i}") for i in range(NCT)]
        # V natural per k-tile; per-head stride 128 elements (256B) keeps
        # the AV stationary loads FWL-aligned; col HD is the ones column
        # (softmax sum), cols HD+1..128 junk (uninitialized: AV writes
        # those psum rows with garbage, which nothing reads). The ones
        # memset is emitted inside the pipeline at each V group.
        VSW = HL * 128
        vS = [persist.tile([128, VSW], F16, name=f"vS{i}", tag=f"vS{i}") for i in range(NTT)]
        attnT = [persist.tile([128, T], F16, name=f"attnT{i}", tag=f"attnT{i}") for i in range(NCT)]

        # weights + biases, fully resident
        wqt = persist.tile([128, NDT, CL], F16, name="wqt", tag="wqt")
        wkt = persist.tile([128, NDT, CL], F16, name="wkt", tag="wkt")
        wvt = persist.tile([128, NDT, CL], F16, name="wvt", tag="wvt")
        wpt = persist.tile([128, NCT, D], F16, name="wpt", tag="wpt")
        bqs = persist.tile([128, NCT], F32, name="bqs", tag="bqs")
        bks = persist.tile([128, NCT], F32, name="bks", tag="bks")
        bvb = persist.tile([128, CL], F32, name="bvb", tag="bvb")
        bpb = persist.tile([128, D], F32, name="bpb", tag="bpb")
        mts = [persist.tile([128, QCW], F16, name=f"mt{i}", tag=f"mt{i}") for i in range(n_pat)]
        def emit_late_loads():
            nc.scalar.dma_start(out=wkt, in_=wk.rearrange("(n p) c -> p n c", p=128))
            nc.scalar.dma_start(out=bks, in_=bk.rearrange("(m p) -> p m", p=128))
            nc.scalar.dma_start(out=wqt, in_=wq.rearrange("(n p) c -> p n c", p=128))
            nc.scalar.dma_start(out=bqs, in_=bq.rearrange("(m p) -> p m", p=128))
            nc.sync.dma_start(out=wpt, in_=wp.rearrange("(n p) c -> p n c", p=128))
            nc.sync.dma_start(
                out=bpb,
                in_=bass.AP(tensor=bp.tensor, offset=bp.offset, ap=[[0, 128]] + list(bp.ap)),
            )
            for i in range(n_pat):
                nc.sync.dma_start(out=mts[i], in_=mp[i])

        # ---- QKV building blocks ----
        xtc_tiles = {}

        xTr = xT.rearrange("(n p) t -> p n t", p=128)

        def emit_xtc(c):
            xTc = xtp.tile([128, NDT, QCW], F16, name="xTc", tag="xTc")
            nc.sync.dma_start(out=xTc, in_=xTr[:, :, c * QCW:(c + 1) * QCW])
            xtc_tiles[c] = xTc

        def emit_group(c, kind, idx):
            """One QKV matmul group for t-chunk c: kind V (t-tile idx) or
            Q/K (c-tile idx)."""
            xTc = xtc_tiles[c]
            tsl = slice(c * QCW, (c + 1) * QCW)
            if kind == "V":
                tt = c * TPC + idx
                if c > 0:
                    nc.gpsimd.memset(
                        vS[tt].rearrange("p (h c) -> p h c", c=128)[:, :, HD:HD + 1], 1.0
                    )
                pv = pqp.tile([128, CL], F32, name="pv", tag="pq")
                for dd in range(NDT):
                    nc.tensor.matmul(
                        pv,
                        lhsT=xTc[:, dd, idx * 128:(idx + 1) * 128],
                        rhs=wvt[:, dd, :],
                        start=(dd == 0),
                        stop=(dd == NDT - 1),
                    )
                nc.vector.tensor_add(
                    vS[tt].rearrange("p (h c) -> p h c", c=128)[:, :, 0:HD],
                    pv.rearrange("p (h d) -> p h d", h=HL),
                    bvb.rearrange("p (h d) -> p h d", h=HL),
                )
            else:
                if kind == "Q":
                    nc.gpsimd.memset(qTe[idx][HD:128, tsl], 0.0)
                    nc.vector.memset(qTo[idx][0:HD, tsl], 0.0)
                wt = wqt if kind == "Q" else wkt
                pb = pqp.tile([128, QCW], F32, name="pb", tag="pq")
                for dd in range(NDT):
                    nc.tensor.matmul(
                        pb,
                        lhsT=wt[:, dd, idx * 128:(idx + 1) * 128],
                        rhs=xTc[:, dd, :],
                        start=(dd == 0),
                        stop=(dd == NDT - 1),
                    )
                if kind == "Q":
                    nc.vector.tensor_scalar_add(
                        qTe[idx][0:HD, tsl], pb[0:HD, :], bqs[0:HD, idx:idx + 1]
                    )
                    nc.vector.tensor_scalar_add(
                        qTo[idx][HD:128, tsl], pb[HD:128, :], bqs[HD:128, idx:idx + 1]
                    )
                else:
                    nc.vector.tensor_scalar_add(
                        kT[idx][:, tsl], pb, bks[:, idx:idx + 1]
                    )

        # ---- attention per (qc, head) ----
        def emit_norm(qc, h, mc, pav):
            """Deferred softmax normalization for a finished head: emitted
            one head late so the broadcast never head-of-line-blocks the
            next head's mask ops on the gpsimd queue."""
            # reciprocal_approx_fast misreads PSUM inputs: bounce l to SBUF
            lrow = rip.tile([1, QCW], F32, name="lrow", tag="lrow")
            nc.vector.tensor_copy(lrow, pav[HD:HD + 1, :])
            rinv = rip.tile([1, QCW], F32, name="rinv", tag="rinv")
            nc.vector.reciprocal_approx_fast(out=rinv, in_=lrow)
            rbs = rbp.tile([HD, QCW], F32, name="rbs", tag="rbs")
            nc.gpsimd.partition_broadcast(rbs, rinv)
            nc.vector.tensor_mul(
                attnT[mc][(h % 2) * HD:(h % 2) * HD + HD, qc * QCW:(qc + 1) * QCW],
                pav[0:HD, :],
                rbs,
            )

        def emit_head(qc, h):
            row = blocks[qc]
            n = len(row)
            mc = h // 2
            qTp = (qTe if h % 2 == 0 else qTo)[mc]
            pav = pvp.tile([128, QCW], F32, name="pav", tag="pav")
            done = 0
            for w0 in range(0, n, WAVE):
                wave = row[w0:w0 + WAVE]
                pts = []
                for (ki, bl, pat) in wave:
                    w = QCW - bl
                    pS = psp.tile([128, QCW], F32, name="pS", tag="pS")
                    nc.tensor.matmul(
                        pS[:, bl:QCW],
                        lhsT=kT[mc][:, ki * 128:(ki + 1) * 128],
                        rhs=qTp[:, qc * QCW + bl:(qc + 1) * QCW],
                        start=True,
                        stop=True,
                    )
                    pT = ptl.tile([128, QCW], F16, name="pT", tag="pT")
                    nc.scalar.activation(pT[:, bl:QCW], pS[:, bl:QCW], AF.Exp, scale=scale)
                    if pat is not None:
                        kind, arg = pat
                        sl = pT[:, bl:QCW]
                        if kind == "tri":
                            # keep where (q - k) >= 0 within the live cols
                            nc.gpsimd.affine_select(
                                out=sl,
                                in_=sl,
                                pattern=[[1, w]],
                                base=arg + bl,
                                channel_multiplier=-1,
                                compare_op=mybir.AluOpType.is_ge,
                                fill=0.0,
                            )
                        else:
                            nc.gpsimd.tensor_mul(sl, sl, mts[arg][:, bl:QCW])
                    pts.append((ki, bl, pT))
                for (ki, bl, pT) in pts:
                    nc.tensor.matmul(
                        pav[:, bl:QCW],
                        lhsT=vS[ki][:, h * 128:h * 128 + 128],
                        rhs=pT[:, bl:QCW],
                        start=(done == 0),
                        stop=(done == n - 1),
                        skip_group_check=True,
                    )
                    done += 1
            emit_norm(qc, h, mc, pav)

        def emit_proj(qc):
            for tv in range(TPC):
                tt = qc * TPC + tv
                yt = ysb.tile([128, D], F32, name="yt", tag="yt")
                for nch in range(NPCH):
                    py = pvp.tile([128, PCH], F32, name="py", tag="pav")
                    for cc in range(NCT):
                        nc.tensor.matmul(
                            py,
                            lhsT=attnT[cc][:, tt * 128:(tt + 1) * 128],
                            rhs=wpt[:, cc, nch * PCH:(nch + 1) * PCH],
                            start=(cc == 0),
                            stop=(cc == NCT - 1),
                        )
                    nc.vector.tensor_add(
                        yt[:, nch * PCH:(nch + 1) * PCH], py,
                        bpb[:, nch * PCH:(nch + 1) * PCH],
                    )
                nc.sync.dma_start(out=y[tt * 128:(tt + 1) * 128, :], in_=yt)

        # ---- emission schedule ----
        # Chunk c's K1..Q3 land inside attention(c)'s early heads (only
        # heads 2mc+ need c-tile mc); V+K0+Q0 of chunk c+1 land in
        # attention(c)'s tail. First chunk primes the pipeline.
        nc.sync.dma_start(out=wvt, in_=wv.rearrange("(n p) c -> p n c", p=128))
        xTc0 = xtp.tile([128, NDT, QCW], F16, name="xTc", tag="xTc")
        nc.scalar.dma_start(out=xTc0, in_=xTr[:, :, 0:QCW])
        xtc_tiles[0] = xTc0
        nc.sync.dma_start(
            out=bvb,
            in_=bass.AP(tensor=bv.tensor, offset=bv.offset, ap=[[0, 128]] + list(bv.ap)),
        )
        emit_late_loads()
        for tt0 in range(TPC):
            nc.gpsimd.memset(
                vS[tt0].rearrange("p (h c) -> p h c", c=128)[:, :, HD:HD + 1], 1.0
            )
        # dummy broadcast: forces the gpsimd custom-op library load during
        # the startup ramp instead of at the first real normalization
        scr1 = persist.tile([1, 8], F32, name="scr1", tag="scr1")
        nc.gpsimd.memset(scr1, 1.0)
        scrb = persist.tile([64, 8], F32, name="scrb", tag="scrb")
        nc.gpsimd.partition_broadcast(scrb, scr1)
        for g in ("V0", "V1", "V2", "V3", "K0", "Q0"):
            emit_group(0, g[0], int(g[1]))
        for qc in range(NQC):
            c_cur, c_nxt = qc, qc + 1
            slots = {
                1: [(c_cur, "K1"), (c_cur, "Q1")] + ([(c_nxt, "V0")] if c_nxt < NQC else []),
                3: [(c_cur, "K2"), (c_cur, "Q2")] + ([(c_nxt, "V1")] if c_nxt < NQC else []),
                5: [(c_cur, "K3"), (c_cur, "Q3")] + ([(c_nxt, "V2")] if c_nxt < NQC else []),
                7: [(c_nxt, "V3"), (c_nxt, "K0"), (c_nxt, "Q0")] if c_nxt < NQC else [],
            }
            for h in range(HL):
                emit_head(qc, h)
                if h == 1 and c_nxt < NQC:
                    emit_xtc(c_nxt)
                for (c, g) in slots.get(h, []):
                    emit_group(c, g[0], int(g[1]))
            emit_proj(qc)
    nc.compile()
    return nc


def classify_mask(mask_bool, T):
    """Classify S^T blocks [k-tile 128, q-chunk QCW] as skip / full /
    tri / pattern, with the first live column per block.

    mask_bool: [T, T] bool, mask_bool[q, k] = attend(q -> k).
    Returns (blocks, patterns, n_pat): blocks[qc] = list of
    (ki, b_live, pat) ordered full-coverage-first."""
    QCW = min(512, T)
    NQC = T // QCW
    NKT = T // 128
    maskT = mask_bool.T  # [k, q]
    patterns = []
    pat_index = {}
    blocks = []
    for qc in range(NQC):
        row = []
        for ki in range(NKT):
            blk = maskT[ki * 128:(ki + 1) * 128, qc * QCW:(qc + 1) * QCW]
            if not blk.any():
                continue
            live_cols = blk.any(axis=0)
            b_live = int(np.argmax(live_cols))
            if not live_cols[b_live:].all():
                b_live = 0  # non-contiguous live region: keep full width
            if blk.all():
                row.append((ki, 0, None))
                continue
            # tril-offset block? keep iff k <= q, i.e. p <= base + f
            base = qc * QCW - ki * 128
            p = np.arange(128)[:, None]
            f = np.arange(QCW)[None, :]
            if np.array_equal(blk, p <= base + f):
                row.append((ki, b_live, ("tri", base)))
                continue
            key = blk.tobytes()
            if key not in pat_index:
                pat_index[key] = len(patterns)
                patterns.append(blk.astype(np.float32))
            row.append((ki, b_live, ("pat", pat_index[key])))
        # full-coverage blocks first so the accumulation start covers
        # every live column of the chunk
        row.sort(key=lambda t: (t[1], t[0]))
        assert row, f"q-chunk {qc} has no active k-tiles"
        assert row[0][1] == 0, f"q-chunk {qc}: no block covers column 0"
        blocks.append(row)
    n_pat = len(patterns)
    pats = np.stack(patterns) if patterns else np.zeros((1, 128, QCW), np.float32)
    return blocks, pats, n_pat


_prog_cache = {}


def _get_program(T, D, HL, mask_bool):
    key = (T, D, HL, mask_bool.tobytes())
    if key not in _prog_cache:
        blocks, pats, n_pat = classify_mask(mask_bool, T)
        nc = build_program(T, D, HL, n_pat, blocks)
        _prog_cache[key] = (nc, blocks, pats)
    return _prog_cache[key]


def kernel(x, W_qkv, b_qkv, W_proj, b_proj, mask):
    out, _ = run_attention(x, W_qkv, b_qkv, W_proj, b_proj, mask)
    return out


def run_attention(x, W_qkv, b_qkv, W_proj, b_proj, mask, trace=False):
    x = np.asarray(x, dtype=np.float32)
    W_qkv = np.asarray(W_qkv, dtype=np.float32)
    b_qkv = np.asarray(b_qkv, dtype=np.float32)
    W_proj = np.asarray(W_proj, dtype=np.float32)
    b_proj = np.asarray(b_proj, dtype=np.float32)
    Bc, T, D = x.shape
    NH = NH_FULL
    HL = NH // 2  # heads per core (two head-groups)
    CL = HL * HD

    mask_bool = np.asarray(mask)[0, 0] != 0

    nc, blocks, pats = _get_program(T, D, HL, mask_bool)

    in_maps = []
    n_cores = 2 * Bc
    for c in range(n_cores):
        b, g = c // 2, c % 2
        sl = slice(g * CL, (g + 1) * CL)
        in_maps.append({
            "xT": np.ascontiguousarray(x[b].T).astype(np.float16),
            "wq": np.ascontiguousarray(W_qkv[:, 0 * D:1 * D][:, sl]).astype(np.float16),
            "wk": np.ascontiguousarray(W_qkv[:, 1 * D:2 * D][:, sl]).astype(np.float16),
            "wv": np.ascontiguousarray(W_qkv[:, 2 * D:3 * D][:, sl]).astype(np.float16),
            "bq": np.ascontiguousarray(b_qkv[0 * D:1 * D][sl]),
            "bk": np.ascontiguousarray(b_qkv[1 * D:2 * D][sl]),
            "bv": np.ascontiguousarray(b_qkv[2 * D:3 * D][sl]),
            "wp": np.ascontiguousarray(W_proj[sl, :]).astype(np.float16),
            "bp": b_proj if g == 0 else np.zeros_like(b_proj),
            "mp": pats.astype(np.float16),
        })

    res = run_bass_kernel_spmd(nc, in_maps, list(range(n_cores)), trace=trace)
    out = np.empty((Bc, T, D), np.float32)
    for b in range(Bc):
        out[b] = res.results[2 * b]["y"] + res.results[2 * b + 1]["y"]
    return out, res


# revision 14
# speedup vs baseline: 1.0051x; 1.0051x over previous
"""Multi-head causal attention (B=4, T=2048, D=1024, 16 heads) on 8 TRN2 cores.

Sharding: core c -> batch b = c//2, head-group g = c%2 (8 of 16 heads).
Each core computes its batch's QKV for its heads, flash-style causal
attention with scores kept transposed (S^T[k, q]; softmax sums come free
via a ones-column appended to V), then a partial output projection
y_part = attn_local @ W_proj[rows]. Host sums the two head-group partials
per batch.

v2 structure: x arrives pre-transposed (host does it for free), all
weights are SBUF-resident, and the per-t-chunk QKV work is emitted
interleaved with the previous q-chunk's attention so the Scalar engine
(exp) and PE overlap across the whole kernel instead of phase-by-phase.
Per head, a wave of score blocks is emitted before the wave's AV
accumulations so the exp latency hides behind the next scores matmul.
Score/exp/mask/AV work is sliced to the live (causal) columns of each
block. Softmax normalization: per-head l row -> reciprocal_approx_fast
-> gpsimd partition_broadcast -> one fused multiply into attnT.
"""

import math
from contextlib import ExitStack

import numpy as np

import concourse.bacc as bacc
import concourse.bass as bass
import concourse.mybir as mybir
import concourse.tile as tile
from concourse.bass_utils import run_bass_kernel_spmd

AF = mybir.ActivationFunctionType
F32 = mybir.dt.float32
F16 = mybir.dt.float16

B_FULL = 4
T_FULL = 2048
D_FULL = 1024
NH_FULL = 16
HD = 64
WAVE = 8  # score blocks per exp/AV wave (bounds live pT tiles)


def build_program(T, D, HL, n_pat, blocks):
    """blocks[qc] = list of (ki, b_live, pat) for active S^T blocks
    [k-tile ki, q-chunk qc]; b_live = first live column within the
    chunk (cols < b_live are fully masked), pat = None | ("tri", base)
    | ("pat", idx)."""
    CL = HL * HD            # local channels (512)
    NDT = D // 128          # contraction tiles for qkv matmuls (8)
    NTT = T // 128          # t-tiles (16)
    QCW = min(512, T)       # q-chunk width
    NQC = T // QCW          # q-chunks (4)
    TPC = QCW // 128        # t-tiles per chunk (4)
    NCT = CL // 128         # c-tiles for q/k/attn storage (4)
    PCH = min(512, D)       # proj output chunk
    NPCH = D // PCH
    scale = 1.0 / math.sqrt(HD)

    nc = bacc.Bacc("TRN2", target_bir_lowering=False, debug=False)
    xT = nc.dram_tensor("xT", [NQC, 128, NDT, QCW], F16, kind="ExternalInput").ap()
    wq = nc.dram_tensor("wq", [128, NDT, CL], F16, kind="ExternalInput").ap()
    wk = nc.dram_tensor("wk", [128, NDT, CL], F16, kind="ExternalInput").ap()
    wv = nc.dram_tensor("wv", [128, NDT, CL], F16, kind="ExternalInput").ap()
    bq = nc.dram_tensor("bq", [CL], F32, kind="ExternalInput").ap()
    bk = nc.dram_tensor("bk", [CL], F32, kind="ExternalInput").ap()
    bv = nc.dram_tensor("bv", [CL], F32, kind="ExternalInput").ap()
    wp = nc.dram_tensor("wp", [128, NCT, D], F16, kind="ExternalInput").ap()
    bp = nc.dram_tensor("bp", [D], F32, kind="ExternalInput").ap()
    mp = nc.dram_tensor("mp", [max(n_pat, 1), 128, QCW], F16, kind="ExternalInput").ap()
    y = nc.dram_tensor("y", [T, D], F32, kind="ExternalOutput").ap()

    with tile.TileContext(nc) as tc, nc.allow_low_precision(
        reason="fp16 operands carry ~1e-3 relative error; tolerance is 2e-2"
    ), ExitStack() as ctx:
        persist = ctx.enter_context(tc.tile_pool(name="persist", bufs=1))
        xtp = ctx.enter_context(tc.tile_pool(name="xtp", bufs=2))
        pqp = ctx.enter_context(tc.tile_pool(name="pqp", bufs=2, space="PSUM"))
        psp = ctx.enter_context(tc.tile_pool(name="psp", bufs=3, space="PSUM"))
        pvp = ctx.enter_context(tc.tile_pool(name="pvp", bufs=3, space="PSUM"))
        ptl = ctx.enter_context(tc.tile_pool(name="ptl", bufs=WAVE + 2))
        rip = ctx.enter_context(tc.tile_pool(name="rip", bufs=3))
        rbp = ctx.enter_context(tc.tile_pool(name="rbp", bufs=3))
        ysb = ctx.enter_context(tc.tile_pool(name="ysb", bufs=3))

        # ---- persistent state ----
        kT = [persist.tile([128, T], F16, name=f"kT{i}", tag=f"kT{i}") for i in range(NCT)]
        # Q^T stored twice, zero-padded per head parity: K<=64 matmuls
        # stream at half rate, so scores contract over all 128 partitions
        # with zero rows killing the other head's channels. The zeroing
        # memsets are emitted per chunk inside the pipeline.
        qTe = [persist.tile([128, T], F16, name=f"qTe{i}", tag=f"qTe{i}") for i in range(NCT)]
        qTo = [persist.tile([128, T], F16, name=f"qTo{i}", tag=f"qTo{i}") for i in range(NCT)]
        # V natural per k-tile; per-head stride 128 elements (256B) keeps
        # the AV stationary loads FWL-aligned; col HD is the ones column
        # (softmax sum), cols HD+1..128 junk (uninitialized: AV writes
        # those psum rows with garbage, which nothing reads). The ones
        # memset is emitted inside the pipeline at each V group.
        VSW = HL * 128
        vS = [persist.tile([128, VSW], F16, name=f"vS{i}", tag=f"vS{i}") for i in range(NTT)]
        attnT = [persist.tile([128, T], F16, name=f"attnT{i}", tag=f"attnT{i}") for i in range(NCT)]

        # weights + biases, fully resident
        wqt = persist.tile([128, NDT, CL], F16, name="wqt", tag="wqt")
        wkt = persist.tile([128, NDT, CL], F16, name="wkt", tag="wkt")
        wvt = persist.tile([128, NDT, CL], F16, name="wvt", tag="wvt")
        wpt = persist.tile([128, NCT, D], F16, name="wpt", tag="wpt")
        bqs = persist.tile([128, NCT], F32, name="bqs", tag="bqs")
        bks = persist.tile([128, NCT], F32, name="bks", tag="bks")
        bvb = persist.tile([128, CL], F32, name="bvb", tag="bvb")
        bpb = persist.tile([128, D], F32, name="bpb", tag="bpb")
        mts = [persist.tile([128, QCW], F16, name=f"mt{i}", tag=f"mt{i}") for i in range(n_pat)]
        def emit_late_loads():
            nc.scalar.dma_start(out=wkt, in_=wk)
            nc.scalar.dma_start(out=bks, in_=bk.rearrange("(m p) -> p m", p=128))
            nc.scalar.dma_start(out=wqt, in_=wq)
            nc.scalar.dma_start(out=bqs, in_=bq.rearrange("(m p) -> p m", p=128))
            nc.sync.dma_start(out=wpt, in_=wp)
            nc.sync.dma_start(
                out=bpb,
                in_=bass.AP(tensor=bp.tensor, offset=bp.offset, ap=[[0, 128]] + list(bp.ap)),
            )
            for i in range(n_pat):
                nc.sync.dma_start(out=mts[i], in_=mp[i])

        # ---- QKV building blocks ----
        xtc_tiles = {}

        def emit_xtc(c):
            xTc = xtp.tile([128, NDT, QCW], F16, name="xTc", tag="xTc")
            nc.sync.dma_start(out=xTc, in_=xT[c])
            xtc_tiles[c] = xTc

        def emit_group(c, kind, idx):
            """One QKV matmul group for t-chunk c: kind V (t-tile idx) or
            Q/K (c-tile idx)."""
            xTc = xtc_tiles[c]
            tsl = slice(c * QCW, (c + 1) * QCW)
            if kind == "V":
                tt = c * TPC + idx
                if c > 0:
                    nc.gpsimd.memset(
                        vS[tt].rearrange("p (h c) -> p h c", c=128)[:, :, HD:HD + 1], 1.0
                    )
                pv = pqp.tile([128, CL], F32, name="pv", tag="pq")
                for dd in range(NDT):
                    nc.tensor.matmul(
                        pv,
                        lhsT=xTc[:, dd, idx * 128:(idx + 1) * 128],
                        rhs=wvt[:, dd, :],
                        start=(dd == 0),
                        stop=(dd == NDT - 1),
                    )
                nc.vector.tensor_add(
                    vS[tt].rearrange("p (h c) -> p h c", c=128)[:, :, 0:HD],
                    pv.rearrange("p (h d) -> p h d", h=HL),
                    bvb.rearrange("p (h d) -> p h d", h=HL),
                )
            else:
                if kind == "Q":
                    nc.gpsimd.memset(qTe[idx][HD:128, tsl], 0.0)
                    nc.vector.memset(qTo[idx][0:HD, tsl], 0.0)
                wt = wqt if kind == "Q" else wkt
                pb = pqp.tile([128, QCW], F32, name="pb", tag="pq")
                for dd in range(NDT):
                    nc.tensor.matmul(
                        pb,
                        lhsT=wt[:, dd, idx * 128:(idx + 1) * 128],
                        rhs=xTc[:, dd, :],
                        start=(dd == 0),
                        stop=(dd == NDT - 1),
                    )
                if kind == "Q":
                    nc.vector.tensor_scalar_add(
                        qTe[idx][0:HD, tsl], pb[0:HD, :], bqs[0:HD, idx:idx + 1]
                    )
                    nc.vector.tensor_scalar_add(
                        qTo[idx][HD:128, tsl], pb[HD:128, :], bqs[HD:128, idx:idx + 1]
                    )
                else:
                    nc.vector.tensor_scalar_add(
                        kT[idx][:, tsl], pb, bks[:, idx:idx + 1]
                    )

        # ---- attention per (qc, head) ----
        def emit_norm(qc, h, mc, pav):
            """Deferred softmax normalization for a finished head: emitted
            one head late so the broadcast never head-of-line-blocks the
            next head's mask ops on the gpsimd queue."""
            # reciprocal_approx_fast misreads PSUM inputs: bounce l to SBUF
            lrow = rip.tile([1, QCW], F32, name="lrow", tag="lrow")
            nc.vector.tensor_copy(lrow, pav[HD:HD + 1, :])
            rinv = rip.tile([1, QCW], F32, name="rinv", tag="rinv")
            nc.vector.reciprocal_approx_fast(out=rinv, in_=lrow)
            rbs = rbp.tile([HD, QCW], F32, name="rbs", tag="rbs")
            nc.gpsimd.partition_broadcast(rbs, rinv)
            nc.vector.tensor_mul(
                attnT[mc][(h % 2) * HD:(h % 2) * HD + HD, qc * QCW:(qc + 1) * QCW],
                pav[0:HD, :],
                rbs,
            )

        def emit_head(qc, h):
            row = blocks[qc]
            n = len(row)
            mc = h // 2
            qTp = (qTe if h % 2 == 0 else qTo)[mc]
            pav = pvp.tile([128, QCW], F32, name="pav", tag="pav")
            done = 0
            for w0 in range(0, n, WAVE):
                wave = row[w0:w0 + WAVE]
                pts = []
                for (ki, bl, pat) in wave:
                    w = QCW - bl
                    pS = psp.tile([128, QCW], F32, name="pS", tag="pS")
                    nc.tensor.matmul(
                        pS[:, bl:QCW],
                        lhsT=kT[mc][:, ki * 128:(ki + 1) * 128],
                        rhs=qTp[:, qc * QCW + bl:(qc + 1) * QCW],
                        start=True,
                        stop=True,
                    )
                    pT = ptl.tile([128, QCW], F16, name="pT", tag="pT")
                    nc.scalar.activation(pT[:, bl:QCW], pS[:, bl:QCW], AF.Exp, scale=scale)
                    if pat is not None:
                        kind, arg = pat
                        sl = pT[:, bl:QCW]
                        if kind == "tri":
                            # keep where (q - k) >= 0 within the live cols
                            nc.gpsimd.affine_select(
                                out=sl,
                                in_=sl,
                                pattern=[[1, w]],
                                base=arg + bl,
                                channel_multiplier=-1,
                                compare_op=mybir.AluOpType.is_ge,
                                fill=0.0,
                            )
                        else:
                            nc.gpsimd.tensor_mul(sl, sl, mts[arg][:, bl:QCW])
                    pts.append((ki, bl, pT))
                for (ki, bl, pT) in pts:
                    nc.tensor.matmul(
                        pav[:, bl:QCW],
                        lhsT=vS[ki][:, h * 128:h * 128 + 128],
                        rhs=pT[:, bl:QCW],
                        start=(done == 0),
                        stop=(done == n - 1),
                        skip_group_check=True,
                    )
                    done += 1
            emit_norm(qc, h, mc, pav)

        def emit_proj(qc):
            for tv in range(TPC):
                tt = qc * TPC + tv
                yt = ysb.tile([128, D], F32, name="yt", tag="yt")
                for nch in range(NPCH):
                    py = pvp.tile([128, PCH], F32, name="py", tag="pav")
                    for cc in range(NCT):
                        nc.tensor.matmul(
                            py,
                            lhsT=attnT[cc][:, tt * 128:(tt + 1) * 128],
                            rhs=wpt[:, cc, nch * PCH:(nch + 1) * PCH],
                            start=(cc == 0),
                            stop=(cc == NCT - 1),
                        )
                    nc.vector.tensor_add(
                        yt[:, nch * PCH:(nch + 1) * PCH], py,
                        bpb[:, nch * PCH:(nch + 1) * PCH],
                    )
                nc.sync.dma_start(out=y[tt * 128:(tt + 1) * 128, :], in_=yt)

        # ---- emission schedule ----
        # Chunk c's K1..Q3 land inside attention(c)'s early heads (only
        # heads 2mc+ need c-tile mc); V+K0+Q0 of chunk c+1 land in
        # attention(c)'s tail. First chunk primes the pipeline.
        nc.sync.dma_start(out=wvt, in_=wv)
        xTc0 = xtp.tile([128, NDT, QCW], F16, name="xTc", tag="xTc")
        nc.scalar.dma_start(out=xTc0, in_=xT[0])
        xtc_tiles[0] = xTc0
        nc.sync.dma_start(
            out=bvb,
            in_=bass.AP(tensor=bv.tensor, offset=bv.offset, ap=[[0, 128]] + list(bv.ap)),
        )
        emit_late_loads()
        for tt0 in range(TPC):
            nc.gpsimd.memset(
                vS[tt0].rearrange("p (h c) -> p h c", c=128)[:, :, HD:HD + 1], 1.0
            )
        # dummy broadcast: forces the gpsimd custom-op library load during
        # the startup ramp instead of at the first real normalization
        scr1 = persist.tile([1, 8], F32, name="scr1", tag="scr1")
        nc.gpsimd.memset(scr1, 1.0)
        scrb = persist.tile([64, 8], F32, name="scrb", tag="scrb")
        nc.gpsimd.partition_broadcast(scrb, scr1)
        for g in ("V0", "V1", "V2", "V3", "K0", "Q0"):
            emit_group(0, g[0], int(g[1]))
        for qc in range(NQC):
            c_cur, c_nxt = qc, qc + 1
            slots = {
                1: [(c_cur, "K1"), (c_cur, "Q1")] + ([(c_nxt, "V0")] if c_nxt < NQC else []),
                3: [(c_cur, "K2"), (c_cur, "Q2")] + ([(c_nxt, "V1")] if c_nxt < NQC else []),
                5: [(c_cur, "K3"), (c_cur, "Q3")] + ([(c_nxt, "V2")] if c_nxt < NQC else []),
                7: [(c_nxt, "V3"), (c_nxt, "K0"), (c_nxt, "Q0")] if c_nxt < NQC else [],
            }
            for h in range(HL):
                emit_head(qc, h)
                if h == 1 and c_nxt < NQC:
                    emit_xtc(c_nxt)
                for (c, g) in slots.get(h, []):
                    emit_group(c, g[0], int(g[1]))
            emit_proj(qc)
    nc.compile()
    return nc


def classify_mask(mask_bool, T):
    """Classify S^T blocks [k-tile 128, q-chunk QCW] as skip / full /
    tri / pattern, with the first live column per block.

    mask_bool: [T, T] bool, mask_bool[q, k] = attend(q -> k).
    Returns (blocks, patterns, n_pat): blocks[qc] = list of
    (ki, b_live, pat) ordered full-coverage-first."""
    QCW = min(512, T)
    NQC = T // QCW
    NKT = T // 128
    maskT = mask_bool.T  # [k, q]
    patterns = []
    pat_index = {}
    blocks = []
    for qc in range(NQC):
        row = []
        for ki in range(NKT):
            blk = maskT[ki * 128:(ki + 1) * 128, qc * QCW:(qc + 1) * QCW]
            if not blk.any():
                continue
            live_cols = blk.any(axis=0)
            b_live = int(np.argmax(live_cols))
            if not live_cols[b_live:].all():
                b_live = 0  # non-contiguous live region: keep full width
            if blk.all():
                row.append((ki, 0, None))
                continue
            # tril-offset block? keep iff k <= q, i.e. p <= base + f
            base = qc * QCW - ki * 128
            p = np.arange(128)[:, None]
            f = np.arange(QCW)[None, :]
            if np.array_equal(blk, p <= base + f):
                row.append((ki, b_live, ("tri", base)))
                continue
            key = blk.tobytes()
            if key not in pat_index:
                pat_index[key] = len(patterns)
                patterns.append(blk.astype(np.float32))
            row.append((ki, b_live, ("pat", pat_index[key])))
        # full-coverage blocks first so the accumulation start covers
        # every live column of the chunk
        row.sort(key=lambda t: (t[1], t[0]))
        assert row, f"q-chunk {qc} has no active k-tiles"
        assert row[0][1] == 0, f"q-chunk {qc}: no block covers column 0"
        blocks.append(row)
    n_pat = len(patterns)
    pats = np.stack(patterns) if patterns else np.zeros((1, 128, QCW), np.float32)
    return blocks, pats, n_pat


_prog_cache = {}


def _get_program(T, D, HL, mask_bool):
    key = (T, D, HL, mask_bool.tobytes())
    if key not in _prog_cache:
        blocks, pats, n_pat = classify_mask(mask_bool, T)
        nc = build_program(T, D, HL, n_pat, blocks)
        _prog_cache[key] = (nc, blocks, pats)
    return _prog_cache[key]


def kernel(x, W_qkv, b_qkv, W_proj, b_proj, mask):
    out, _ = run_attention(x, W_qkv, b_qkv, W_proj, b_proj, mask)
    return out


def run_attention(x, W_qkv, b_qkv, W_proj, b_proj, mask, trace=False):
    x = np.asarray(x, dtype=np.float32)
    W_qkv = np.asarray(W_qkv, dtype=np.float32)
    b_qkv = np.asarray(b_qkv, dtype=np.float32)
    W_proj = np.asarray(W_proj, dtype=np.float32)
    b_proj = np.asarray(b_proj, dtype=np.float32)
    Bc, T, D = x.shape
    NH = NH_FULL
    HL = NH // 2  # heads per core (two head-groups)
    CL = HL * HD

    mask_bool = np.asarray(mask)[0, 0] != 0

    nc, blocks, pats = _get_program(T, D, HL, mask_bool)

    NDT = D // 128
    NCT = CL // 128
    QCW = min(512, T)
    NQC = T // QCW

    def tile_w(w):
        # [D, CL] -> [128, NDT, CL] with (p, n, c) = w[n*128+p, c]
        return np.ascontiguousarray(
            w.astype(np.float16).reshape(NDT, 128, -1).transpose(1, 0, 2))

    in_maps = []
    n_cores = 2 * Bc
    for c in range(n_cores):
        b, g = c // 2, c % 2
        sl = slice(g * CL, (g + 1) * CL)
        xt16 = x[b].T.astype(np.float16).reshape(NDT, 128, T)
        xtt = np.ascontiguousarray(
            xt16.reshape(NDT, 128, NQC, QCW).transpose(2, 1, 0, 3))
        in_maps.append({
            "xT": xtt,
            "wq": tile_w(W_qkv[:, 0 * D:1 * D][:, sl]),
            "wk": tile_w(W_qkv[:, 1 * D:2 * D][:, sl]),
            "wv": tile_w(W_qkv[:, 2 * D:3 * D][:, sl]),
            "bq": np.ascontiguousarray(b_qkv[0 * D:1 * D][sl]),
            "bk": np.ascontiguousarray(b_qkv[1 * D:2 * D][sl]),
            "bv": np.ascontiguousarray(b_qkv[2 * D:3 * D][sl]),
            "wp": np.ascontiguousarray(
                W_proj[sl, :].astype(np.float16).reshape(NCT, 128, D).transpose(1, 0, 2)),
            "bp": b_proj if g == 0 else np.zeros_like(b_proj),
            "mp": pats.astype(np.float16),
        })

    res = run_bass_kernel_spmd(nc, in_maps, list(range(n_cores)), trace=trace)
    out = np.empty((Bc, T, D), np.float32)
    for b in range(Bc):
        out[b] = res.results[2 * b]["y"] + res.results[2 * b + 1]["y"]
    return out, res


# revision 15
# speedup vs baseline: 1.0197x; 1.0145x over previous
"""Multi-head causal attention (B=4, T=2048, D=1024, 16 heads) on 8 TRN2 cores.

Sharding: core c -> batch b = c//2, head-group g = c%2 (8 of 16 heads).
Each core computes its batch's QKV for its heads, flash-style causal
attention with scores kept transposed (S^T[k, q]; softmax sums come free
via a ones-column appended to V), then a partial output projection
y_part = attn_local @ W_proj[rows]. Host sums the two head-group partials
per batch.

v2 structure: x arrives pre-transposed (host does it for free), all
weights are SBUF-resident, and the per-t-chunk QKV work is emitted
interleaved with the previous q-chunk's attention so the Scalar engine
(exp) and PE overlap across the whole kernel instead of phase-by-phase.
Per head, a wave of score blocks is emitted before the wave's AV
accumulations so the exp latency hides behind the next scores matmul.
Score/exp/mask/AV work is sliced to the live (causal) columns of each
block. Softmax normalization: per-head l row -> reciprocal_approx_fast
-> gpsimd partition_broadcast -> one fused multiply into attnT.
"""

import math
from contextlib import ExitStack

import numpy as np

import concourse.bacc as bacc
import concourse.bass as bass
import concourse.mybir as mybir
import concourse.tile as tile
from concourse.bass_utils import run_bass_kernel_spmd

AF = mybir.ActivationFunctionType
F32 = mybir.dt.float32
F16 = mybir.dt.float16

B_FULL = 4
T_FULL = 2048
D_FULL = 1024
NH_FULL = 16
HD = 64
WAVE = 8  # score blocks per exp/AV wave (bounds live pT tiles)


def build_program(T, D, HL, n_pat, blocks):
    """blocks[qc] = list of (ki, b_live, pat) for active S^T blocks
    [k-tile ki, q-chunk qc]; b_live = first live column within the
    chunk (cols < b_live are fully masked), pat = None | ("tri", base)
    | ("pat", idx)."""
    CL = HL * HD            # local channels (512)
    NDT = D // 128          # contraction tiles for qkv matmuls (8)
    NTT = T // 128          # t-tiles (16)
    QCW = min(512, T)       # q-chunk width
    NQC = T // QCW          # q-chunks (4)
    TPC = QCW // 128        # t-tiles per chunk (4)
    NCT = CL // 128         # c-tiles for q/k/attn storage (4)
    PCH = min(512, D)       # proj output chunk
    NPCH = D // PCH
    scale = 1.0 / math.sqrt(HD)

    nc = bacc.Bacc("TRN2", target_bir_lowering=False, debug=False)
    xT = nc.dram_tensor("xT", [NQC, 128, NDT, QCW], F16, kind="ExternalInput").ap()
    wq = nc.dram_tensor("wq", [128, NDT, CL], F16, kind="ExternalInput").ap()
    wk = nc.dram_tensor("wk", [128, NDT, CL], F16, kind="ExternalInput").ap()
    wv = nc.dram_tensor("wv", [128, NDT, CL], F16, kind="ExternalInput").ap()
    bq = nc.dram_tensor("bq", [CL], F32, kind="ExternalInput").ap()
    bk = nc.dram_tensor("bk", [CL], F32, kind="ExternalInput").ap()
    bv = nc.dram_tensor("bv", [CL], F32, kind="ExternalInput").ap()
    wp = nc.dram_tensor("wp", [128, NCT, D], F16, kind="ExternalInput").ap()
    bp = nc.dram_tensor("bp", [D], F32, kind="ExternalInput").ap()
    mp = nc.dram_tensor("mp", [max(n_pat, 1), 128, QCW], F16, kind="ExternalInput").ap()
    y = nc.dram_tensor("y", [T, D], F16, kind="ExternalOutput").ap()

    with tile.TileContext(nc) as tc, nc.allow_low_precision(
        reason="fp16 operands carry ~1e-3 relative error; tolerance is 2e-2"
    ), ExitStack() as ctx:
        persist = ctx.enter_context(tc.tile_pool(name="persist", bufs=1))
        xtp = ctx.enter_context(tc.tile_pool(name="xtp", bufs=2))
        pqp = ctx.enter_context(tc.tile_pool(name="pqp", bufs=2, space="PSUM"))
        psp = ctx.enter_context(tc.tile_pool(name="psp", bufs=3, space="PSUM"))
        pvp = ctx.enter_context(tc.tile_pool(name="pvp", bufs=3, space="PSUM"))
        ptl = ctx.enter_context(tc.tile_pool(name="ptl", bufs=WAVE + 2))
        rip = ctx.enter_context(tc.tile_pool(name="rip", bufs=3))
        rbp = ctx.enter_context(tc.tile_pool(name="rbp", bufs=3))
        ysb = ctx.enter_context(tc.tile_pool(name="ysb", bufs=3))

        # ---- persistent state ----
        kT = [persist.tile([128, T], F16, name=f"kT{i}", tag=f"kT{i}") for i in range(NCT)]
        # Q^T stored twice, zero-padded per head parity: K<=64 matmuls
        # stream at half rate, so scores contract over all 128 partitions
        # with zero rows killing the other head's channels. The zeroing
        # memsets are emitted per chunk inside the pipeline.
        qTe = [persist.tile([128, T], F16, name=f"qTe{i}", tag=f"qTe{i}") for i in range(NCT)]
        qTo = [persist.tile([128, T], F16, name=f"qTo{i}", tag=f"qTo{i}") for i in range(NCT)]
        # V natural per k-tile; per-head stride 128 elements (256B) keeps
        # the AV stationary loads FWL-aligned; col HD is the ones column
        # (softmax sum), cols HD+1..128 junk (uninitialized: AV writes
        # those psum rows with garbage, which nothing reads). The ones
        # memset is emitted inside the pipeline at each V group.
        VSW = HL * 128
        vS = [persist.tile([128, VSW], F16, name=f"vS{i}", tag=f"vS{i}") for i in range(NTT)]
        attnT = [persist.tile([128, T], F16, name=f"attnT{i}", tag=f"attnT{i}") for i in range(NCT)]

        # weights + biases, fully resident
        wqt = persist.tile([128, NDT, CL], F16, name="wqt", tag="wqt")
        wkt = persist.tile([128, NDT, CL], F16, name="wkt", tag="wkt")
        wvt = persist.tile([128, NDT, CL], F16, name="wvt", tag="wvt")
        wpt = persist.tile([128, NCT, D], F16, name="wpt", tag="wpt")
        bqs = persist.tile([128, NCT], F32, name="bqs", tag="bqs")
        bks = persist.tile([128, NCT], F32, name="bks", tag="bks")
        bvb = persist.tile([128, CL], F32, name="bvb", tag="bvb")
        bpb = persist.tile([128, D], F32, name="bpb", tag="bpb")
        mts = [persist.tile([128, QCW], F16, name=f"mt{i}", tag=f"mt{i}") for i in range(n_pat)]
        def emit_late_loads():
            nc.scalar.dma_start(out=wkt, in_=wk)
            nc.scalar.dma_start(out=bks, in_=bk.rearrange("(m p) -> p m", p=128))
            nc.scalar.dma_start(out=wqt, in_=wq)
            nc.scalar.dma_start(out=bqs, in_=bq.rearrange("(m p) -> p m", p=128))
            nc.sync.dma_start(out=wpt, in_=wp)
            nc.sync.dma_start(
                out=bpb,
                in_=bass.AP(tensor=bp.tensor, offset=bp.offset, ap=[[0, 128]] + list(bp.ap)),
            )
            for i in range(n_pat):
                nc.sync.dma_start(out=mts[i], in_=mp[i])

        # ---- QKV building blocks ----
        xtc_tiles = {}

        def emit_xtc(c):
            xTc = xtp.tile([128, NDT, QCW], F16, name="xTc", tag="xTc")
            nc.sync.dma_start(out=xTc, in_=xT[c])
            xtc_tiles[c] = xTc

        def emit_group(c, kind, idx):
            """One QKV matmul group for t-chunk c: kind V (t-tile idx) or
            Q/K (c-tile idx)."""
            xTc = xtc_tiles[c]
            tsl = slice(c * QCW, (c + 1) * QCW)
            if kind == "V":
                tt = c * TPC + idx
                if c > 0:
                    nc.gpsimd.memset(
                        vS[tt].rearrange("p (h c) -> p h c", c=128)[:, :, HD:HD + 1], 1.0
                    )
                pv = pqp.tile([128, CL], F32, name="pv", tag="pq")
                for dd in range(NDT):
                    nc.tensor.matmul(
                        pv,
                        lhsT=xTc[:, dd, idx * 128:(idx + 1) * 128],
                        rhs=wvt[:, dd, :],
                        start=(dd == 0),
                        stop=(dd == NDT - 1),
                    )
                nc.vector.tensor_add(
                    vS[tt].rearrange("p (h c) -> p h c", c=128)[:, :, 0:HD],
                    pv.rearrange("p (h d) -> p h d", h=HL),
                    bvb.rearrange("p (h d) -> p h d", h=HL),
                )
            else:
                if kind == "Q":
                    nc.gpsimd.memset(qTe[idx][HD:128, tsl], 0.0)
                    nc.vector.memset(qTo[idx][0:HD, tsl], 0.0)
                wt = wqt if kind == "Q" else wkt
                pb = pqp.tile([128, QCW], F32, name="pb", tag="pq")
                for dd in range(NDT):
                    nc.tensor.matmul(
                        pb,
                        lhsT=wt[:, dd, idx * 128:(idx + 1) * 128],
                        rhs=xTc[:, dd, :],
                        start=(dd == 0),
                        stop=(dd == NDT - 1),
                    )
                if kind == "Q":
                    nc.vector.tensor_scalar_add(
                        qTe[idx][0:HD, tsl], pb[0:HD, :], bqs[0:HD, idx:idx + 1]
                    )
                    nc.vector.tensor_scalar_add(
                        qTo[idx][HD:128, tsl], pb[HD:128, :], bqs[HD:128, idx:idx + 1]
                    )
                else:
                    nc.vector.tensor_scalar_add(
                        kT[idx][:, tsl], pb, bks[:, idx:idx + 1]
                    )

        # ---- attention per (qc, head) ----
        def emit_norm_a(pav):
            """1/l for a finished head: l -> SBUF -> approx reciprocal ->
            broadcast across 64 partitions."""
            # reciprocal_approx_fast misreads PSUM inputs: bounce l to SBUF
            lrow = rip.tile([1, QCW], F32, name="lrow", tag="lrow")
            nc.vector.tensor_copy(lrow, pav[HD:HD + 1, :])
            rinv = rip.tile([1, QCW], F32, name="rinv", tag="rinv")
            nc.vector.reciprocal_approx_fast(out=rinv, in_=lrow)
            rbs = rbp.tile([HD, QCW], F32, name="rbs", tag="rbs")
            nc.gpsimd.partition_broadcast(rbs, rinv)
            return rbs

        def emit_norm_b(qc, h, mc, pav, rbs):
            """The attnT multiply, deferred one head so consecutive norm
            chains overlap instead of serializing on the DVE queue."""
            nc.vector.tensor_mul(
                attnT[mc][(h % 2) * HD:(h % 2) * HD + HD, qc * QCW:(qc + 1) * QCW],
                pav[0:HD, :],
                rbs,
            )

        def emit_head(qc, h):
            row = blocks[qc]
            n = len(row)
            mc = h // 2
            qTp = (qTe if h % 2 == 0 else qTo)[mc]
            pav = pvp.tile([128, QCW], F32, name="pav", tag="pav")
            done = 0
            for w0 in range(0, n, WAVE):
                wave = row[w0:w0 + WAVE]
                pts = []
                for (ki, bl, pat) in wave:
                    w = QCW - bl
                    pS = psp.tile([128, QCW], F32, name="pS", tag="pS")
                    nc.tensor.matmul(
                        pS[:, bl:QCW],
                        lhsT=kT[mc][:, ki * 128:(ki + 1) * 128],
                        rhs=qTp[:, qc * QCW + bl:(qc + 1) * QCW],
                        start=True,
                        stop=True,
                    )
                    pT = ptl.tile([128, QCW], F16, name="pT", tag="pT")
                    nc.scalar.activation(pT[:, bl:QCW], pS[:, bl:QCW], AF.Exp, scale=scale)
                    if pat is not None:
                        kind, arg = pat
                        sl = pT[:, bl:QCW]
                        if kind == "tri":
                            # keep where (q - k) >= 0 within the live cols
                            nc.gpsimd.affine_select(
                                out=sl,
                                in_=sl,
                                pattern=[[1, w]],
                                base=arg + bl,
                                channel_multiplier=-1,
                                compare_op=mybir.AluOpType.is_ge,
                                fill=0.0,
                            )
                        else:
                            nc.gpsimd.tensor_mul(sl, sl, mts[arg][:, bl:QCW])
                    pts.append((ki, bl, pT))
                for (ki, bl, pT) in pts:
                    nc.tensor.matmul(
                        pav[:, bl:QCW],
                        lhsT=vS[ki][:, h * 128:h * 128 + 128],
                        rhs=pT[:, bl:QCW],
                        start=(done == 0),
                        stop=(done == n - 1),
                        skip_group_check=True,
                    )
                    done += 1
            return mc, pav

        def emit_proj(qc):
            for tv in range(TPC):
                tt = qc * TPC + tv
                yt = ysb.tile([128, D], F16, name="yt", tag="yt")
                for nch in range(NPCH):
                    py = pvp.tile([128, PCH], F32, name="py", tag="pav")
                    for cc in range(NCT):
                        nc.tensor.matmul(
                            py,
                            lhsT=attnT[cc][:, tt * 128:(tt + 1) * 128],
                            rhs=wpt[:, cc, nch * PCH:(nch + 1) * PCH],
                            start=(cc == 0),
                            stop=(cc == NCT - 1),
                        )
                    nc.vector.tensor_add(
                        yt[:, nch * PCH:(nch + 1) * PCH], py,
                        bpb[:, nch * PCH:(nch + 1) * PCH],
                    )
                nc.sync.dma_start(out=y[tt * 128:(tt + 1) * 128, :], in_=yt)

        # ---- emission schedule ----
        # Chunk c's K1..Q3 land inside attention(c)'s early heads (only
        # heads 2mc+ need c-tile mc); V+K0+Q0 of chunk c+1 land in
        # attention(c)'s tail. First chunk primes the pipeline.
        nc.sync.dma_start(out=wvt, in_=wv)
        xTc0 = xtp.tile([128, NDT, QCW], F16, name="xTc", tag="xTc")
        nc.scalar.dma_start(out=xTc0, in_=xT[0])
        xtc_tiles[0] = xTc0
        nc.sync.dma_start(
            out=bvb,
            in_=bass.AP(tensor=bv.tensor, offset=bv.offset, ap=[[0, 128]] + list(bv.ap)),
        )
        emit_late_loads()
        for tt0 in range(TPC):
            nc.gpsimd.memset(
                vS[tt0].rearrange("p (h c) -> p h c", c=128)[:, :, HD:HD + 1], 1.0
            )
        # dummy broadcast: forces the gpsimd custom-op library load during
        # the startup ramp instead of at the first real normalization
        scr1 = persist.tile([1, 8], F32, name="scr1", tag="scr1")
        nc.gpsimd.memset(scr1, 1.0)
        scrb = persist.tile([64, 8], F32, name="scrb", tag="scrb")
        nc.gpsimd.partition_broadcast(scrb, scr1)
        for g in ("V0", "V1", "V2", "V3", "K0", "Q0"):
            emit_group(0, g[0], int(g[1]))
        pending_mul = None
        for qc in range(NQC):
            c_cur, c_nxt = qc, qc + 1
            slots = {
                1: [(c_cur, "K1"), (c_cur, "Q1")] + ([(c_nxt, "V0")] if c_nxt < NQC else []),
                3: [(c_cur, "K2"), (c_cur, "Q2")] + ([(c_nxt, "V1")] if c_nxt < NQC else []),
                5: [(c_cur, "K3"), (c_cur, "Q3")] + ([(c_nxt, "V2")] if c_nxt < NQC else []),
                7: [(c_nxt, "V3"), (c_nxt, "K0"), (c_nxt, "Q0")] if c_nxt < NQC else [],
            }
            for h in range(HL):
                mc, pav = emit_head(qc, h)
                if pending_mul is not None:
                    emit_norm_b(*pending_mul)
                rbs = emit_norm_a(pav)
                pending_mul = (qc, h, mc, pav, rbs)
                if h == 1 and c_nxt < NQC:
                    emit_xtc(c_nxt)
                for (c, g) in slots.get(h, []):
                    emit_group(c, g[0], int(g[1]))
            emit_norm_b(*pending_mul)
            pending_mul = None
            emit_proj(qc)
    nc.compile()
    return nc


def classify_mask(mask_bool, T):
    """Classify S^T blocks [k-tile 128, q-chunk QCW] as skip / full /
    tri / pattern, with the first live column per block.

    mask_bool: [T, T] bool, mask_bool[q, k] = attend(q -> k).
    Returns (blocks, patterns, n_pat): blocks[qc] = list of
    (ki, b_live, pat) ordered full-coverage-first."""
    QCW = min(512, T)
    NQC = T // QCW
    NKT = T // 128
    maskT = mask_bool.T  # [k, q]
    patterns = []
    pat_index = {}
    blocks = []
    for qc in range(NQC):
        row = []
        for ki in range(NKT):
            blk = maskT[ki * 128:(ki + 1) * 128, qc * QCW:(qc + 1) * QCW]
            if not blk.any():
                continue
            live_cols = blk.any(axis=0)
            b_live = int(np.argmax(live_cols))
            if not live_cols[b_live:].all():
                b_live = 0  # non-contiguous live region: keep full width
            if blk.all():
                row.append((ki, 0, None))
                continue
            # tril-offset block? keep iff k <= q, i.e. p <= base + f
            base = qc * QCW - ki * 128
            p = np.arange(128)[:, None]
            f = np.arange(QCW)[None, :]
            if np.array_equal(blk, p <= base + f):
                row.append((ki, b_live, ("tri", base)))
                continue
            key = blk.tobytes()
            if key not in pat_index:
                pat_index[key] = len(patterns)
                patterns.append(blk.astype(np.float32))
            row.append((ki, b_live, ("pat", pat_index[key])))
        # full-coverage blocks first so the accumulation start covers
        # every live column of the chunk
        row.sort(key=lambda t: (t[1], t[0]))
        assert row, f"q-chunk {qc} has no active k-tiles"
        assert row[0][1] == 0, f"q-chunk {qc}: no block covers column 0"
        blocks.append(row)
    n_pat = len(patterns)
    pats = np.stack(patterns) if patterns else np.zeros((1, 128, QCW), np.float32)
    return blocks, pats, n_pat


_prog_cache = {}


def _get_program(T, D, HL, mask_bool):
    key = (T, D, HL, mask_bool.tobytes())
    if key not in _prog_cache:
        blocks, pats, n_pat = classify_mask(mask_bool, T)
        nc = build_program(T, D, HL, n_pat, blocks)
        _prog_cache[key] = (nc, blocks, pats)
    return _prog_cache[key]


def kernel(x, W_qkv, b_qkv, W_proj, b_proj, mask):
    out, _ = run_attention(x, W_qkv, b_qkv, W_proj, b_proj, mask)
    return out


def run_attention(x, W_qkv, b_qkv, W_proj, b_proj, mask, trace=False):
    x = np.asarray(x, dtype=np.float32)
    W_qkv = np.asarray(W_qkv, dtype=np.float32)
    b_qkv = np.asarray(b_qkv, dtype=np.float32)
    W_proj = np.asarray(W_proj, dtype=np.float32)
    b_proj = np.asarray(b_proj, dtype=np.float32)
    Bc, T, D = x.shape
    NH = NH_FULL
    HL = NH // 2  # heads per core (two head-groups)
    CL = HL * HD

    mask_bool = np.asarray(mask)[0, 0] != 0

    nc, blocks, pats = _get_program(T, D, HL, mask_bool)

    NDT = D // 128
    NCT = CL // 128
    QCW = min(512, T)
    NQC = T // QCW

    def tile_w(w):
        # [D, CL] -> [128, NDT, CL] with (p, n, c) = w[n*128+p, c]
        return np.ascontiguousarray(
            w.astype(np.float16).reshape(NDT, 128, -1).transpose(1, 0, 2))

    in_maps = []
    n_cores = 2 * Bc
    for c in range(n_cores):
        b, g = c // 2, c % 2
        sl = slice(g * CL, (g + 1) * CL)
        xt16 = x[b].T.astype(np.float16).reshape(NDT, 128, T)
        xtt = np.ascontiguousarray(
            xt16.reshape(NDT, 128, NQC, QCW).transpose(2, 1, 0, 3))
        in_maps.append({
            "xT": xtt,
            "wq": tile_w(W_qkv[:, 0 * D:1 * D][:, sl]),
            "wk": tile_w(W_qkv[:, 1 * D:2 * D][:, sl]),
            "wv": tile_w(W_qkv[:, 2 * D:3 * D][:, sl]),
            "bq": np.ascontiguousarray(b_qkv[0 * D:1 * D][sl]),
            "bk": np.ascontiguousarray(b_qkv[1 * D:2 * D][sl]),
            "bv": np.ascontiguousarray(b_qkv[2 * D:3 * D][sl]),
            "wp": np.ascontiguousarray(
                W_proj[sl, :].astype(np.float16).reshape(NCT, 128, D).transpose(1, 0, 2)),
            "bp": b_proj if g == 0 else np.zeros_like(b_proj),
            "mp": pats.astype(np.float16),
        })

    res = run_bass_kernel_spmd(nc, in_maps, list(range(n_cores)), trace=trace)
    out = np.empty((Bc, T, D), np.float32)
    for b in range(Bc):
        out[b] = (res.results[2 * b]["y"].astype(np.float32)
                  + res.results[2 * b + 1]["y"].astype(np.float32))
    return out, res
